# revision 10
# baseline (speedup 1.0000x reference)
"""BilinearPooling Trainium2 kernel.

reference:
    pooled = audio * rowsum(visual)            # [B, D]
    out    = pooled / max(||pooled||_2, eps)   # L2 normalize along D

Since pooled[b,:] = audio[b,:] * s_b with s_b = rowsum(visual[b,:]),
||pooled_b|| = |s_b| * ||audio_b||, so

    out[b,:] = audio[b,:] * s_b / max(|s_b| * ||audio_b||, eps)

Pure data parallel across 8 NeuronCores: batch 8192 -> 1024 rows/core.
Per core: 8 row-tiles of [128, 2048] f32, each processed as two 1024-col
halves so compute starts as soon as half a tile has landed and the
end-of-stream drain chain is short. Three DMA queues carry the three
equal traffic streams concurrently: audio loads on the SP HWDGE ring,
visual loads on the SWDGE (gpsimd) ring, stores on the ACT HWDGE ring.
Engine split: rowsum halves + tiny scale chain + odd-half muls on DVE,
square+accumulate halves + sqrt + even-half muls on ACT; GpSimd only
issues DMA descriptors (its tensor ops are ~14x slower than DVE here).
Memory-bound: 24 MiB/core, measured ~400 GB/s DMA when uncontended.
"""

import numpy as np

import concourse.bass as bass
import concourse.tile as tile
from concourse import mybir
from concourse.bacc import Bacc
from concourse.bass_utils import run_bass_kernel_spmd

B, D = 8192, 2048
N_CORES = 8
ROWS = B // N_CORES          # 1024 rows per core
P = 128                      # SBUF partitions
N_TILES = ROWS // P          # 8
D2 = D // 2                  # column half
EPS = 1e-12
FP32 = mybir.dt.float32


def build_bass():
    # Bacc (not raw Bass): its finalize() runs the compile pipeline that
    # splits multi-wait instructions into event-semaphore chains — TRN2
    # allows at most one sync wait per instruction, and Tile freely emits
    # more ("Too many sync wait commands" from walrus otherwise).
    nc = Bacc()
    audio = nc.declare_dram_parameter("audio", [ROWS, D], FP32, isOutput=False)
    visual = nc.declare_dram_parameter("visual", [ROWS, D], FP32, isOutput=False)
    out = nc.declare_dram_parameter("out", [ROWS, D], FP32, isOutput=True)

    with tile.TileContext(nc) as tc:
        with (
            tc.tile_pool(name="a_pool", bufs=8) as a_pool,
            tc.tile_pool(name="v_pool", bufs=8) as v_pool,
            tc.tile_pool(name="stats", bufs=8) as stats,
            tc.tile_pool(name="singles", bufs=1) as singles,
        ):
            zero = singles.tile([P, 1], FP32)
            nc.vector.memset(zero, 0.0)

            for t in range(N_TILES):
                r0 = t * P
                ah = []
                vh = []
                sh = stats.tile([P, 2], FP32)
                qh = stats.tile([P, 2], FP32)
                for h in range(2):
                    c0 = h * D2
                    a = a_pool.tile([P, D2], FP32)
                    v = v_pool.tile([P, D2], FP32)
                    nc.sync.dma_start(out=a, in_=audio[r0 : r0 + P, c0 : c0 + D2])
                    nc.gpsimd.dma_start(out=v, in_=visual[r0 : r0 + P, c0 : c0 + D2])
                    ah.append(a)
                    vh.append(v)
                    # s half on DVE; audio^2 rowsum half on ACT (full-size
                    # squares land in the dead visual half as scratch).
                    nc.vector.reduce_sum(
                        out=sh[:, h : h + 1], in_=v, axis=mybir.AxisListType.X
                    )
                    nc.scalar.activation(
                        out=v,
                        in_=a,
                        func=mybir.ActivationFunctionType.Square,
                        bias=zero,
                        accum_out=qh[:, h : h + 1],
                    )

                # sc = s / max(sqrt(s^2 * asq), eps)
                s = stats.tile([P, 1], FP32)
                nc.vector.tensor_add(out=s, in0=sh[:, 0:1], in1=sh[:, 1:2])
                nrm = stats.tile([P, 1], FP32)
                nc.vector.tensor_add(out=nrm, in0=qh[:, 0:1], in1=qh[:, 1:2])
                nc.vector.tensor_mul(out=nrm, in0=nrm, in1=s)
                nc.vector.tensor_mul(out=nrm, in0=nrm, in1=s)
                nc.scalar.activation(
                    out=nrm,
                    in_=nrm,
                    func=mybir.ActivationFunctionType.Sqrt,
                    bias=zero,
                )
                nc.vector.tensor_scalar_max(out=nrm, in0=nrm, scalar1=EPS)
                nc.vector.reciprocal(out=nrm, in_=nrm)
                sc = stats.tile([P, 1], FP32)
                nc.vector.tensor_mul(out=sc, in0=s, in1=nrm)

                # out halves = audio halves * sc, in place, on two engines
                # in parallel; store each half as soon as its mul is done,
                # alternating the two HWDGE rings.
                nc.scalar.mul(out=ah[0], in_=ah[0], mul=sc)
                nc.vector.tensor_scalar_mul(out=ah[1], in0=ah[1], scalar1=sc)
                nc.scalar.dma_start(out=out[r0 : r0 + P, 0:D2], in_=ah[0])
                nc.sync.dma_start(out=out[r0 : r0 + P, D2:D], in_=ah[1])

    # Runs Bacc.compile() (event-sem wait splitting, reg alloc, ISA
    # codegen) and freezes; run_bass_via_pjrt requires a finalized module.
    nc.finalize()
    return nc


_NC = None


def _get_nc():
    global _NC
    if _NC is None:
        _NC = build_bass()
    return _NC


def kernel(audio: np.ndarray, visual: np.ndarray) -> np.ndarray:
    audio = np.ascontiguousarray(audio, dtype=np.float32)
    visual = np.ascontiguousarray(visual, dtype=np.float32)
    nc = _get_nc()
    in_maps = [
        {
            "audio": audio[i * ROWS : (i + 1) * ROWS],
            "visual": visual[i * ROWS : (i + 1) * ROWS],
        }
        for i in range(N_CORES)
    ]
    res = run_bass_kernel_spmd(nc, in_maps, core_ids=list(range(N_CORES)))
    return np.concatenate([r["out"] for r in res.results], axis=0)


# revision 11
# speedup vs baseline: 1.1513x; 1.1513x over previous
"""BilinearPooling Trainium2 kernel.

reference:
    pooled = audio * rowsum(visual)            # [B, D]
    out    = pooled / max(||pooled||_2, eps)   # L2 normalize along D

Since pooled[b,:] = audio[b,:] * s_b with s_b = rowsum(visual[b,:]),
||pooled_b|| = |s_b| * ||audio_b||, so

    out[b,:] = audio[b,:] * s_b / max(|s_b| * ||audio_b||, eps)

Pure data parallel across 8 NeuronCores: batch 8192 -> 1024 rows/core.
Per core: 8 row-tiles of [128, 2048] f32, each processed as two 1024-col
halves so compute starts as soon as half a tile has landed and the
end-of-stream drain chain is short. Three DMA queues carry the three
equal traffic streams concurrently: audio loads on the SP HWDGE ring,
visual loads on the SWDGE (gpsimd) ring, stores on the ACT HWDGE ring.
Engine split: rowsum halves + tiny scale chain + odd-half muls on DVE,
square+accumulate halves + sqrt + even-half muls on ACT; GpSimd only
issues DMA descriptors (its tensor ops are ~14x slower than DVE here).
Memory-bound: 24 MiB/core, measured ~400 GB/s DMA when uncontended.
"""

import numpy as np

import concourse.bass as bass
import concourse.tile as tile
from concourse import mybir
from concourse.bacc import Bacc
from concourse.bass_utils import run_bass_kernel_spmd

B, D = 8192, 2048
N_CORES = 8
ROWS = B // N_CORES          # 1024 rows per core
P = 128                      # SBUF partitions
N_TILES = ROWS // P          # 8
D2 = D // 2                  # column half
EPS = 1e-12
FP32 = mybir.dt.float32


def build_bass():
    # Bacc (not raw Bass): its finalize() runs the compile pipeline that
    # splits multi-wait instructions into event-semaphore chains — TRN2
    # allows at most one sync wait per instruction, and Tile freely emits
    # more ("Too many sync wait commands" from walrus otherwise).
    nc = Bacc()
    audio = nc.declare_dram_parameter("audio", [ROWS, D], FP32, isOutput=False)
    visual = nc.declare_dram_parameter("visual", [ROWS, D], FP32, isOutput=False)
    out = nc.declare_dram_parameter("out", [ROWS, D], FP32, isOutput=True)

    with tile.TileContext(nc) as tc:
        with (
            tc.tile_pool(name="a_pool", bufs=8) as a_pool,
            tc.tile_pool(name="v_pool", bufs=8) as v_pool,
            tc.tile_pool(name="stats", bufs=8) as stats,
            tc.tile_pool(name="singles", bufs=1) as singles,
        ):
            zero = singles.tile([P, 1], FP32)
            nc.vector.memset(zero, 0.0)

            for t in range(N_TILES):
                r0 = t * P
                ah = []
                vh = []
                sh = stats.tile([P, 2], FP32)
                qh = stats.tile([P, 2], FP32)
                for h in range(2):
                    c0 = h * D2
                    a = a_pool.tile([P, D2], FP32)
                    v = v_pool.tile([P, D2], FP32)
                    # Both loads on the SP HWDGE ring: SWDGE (gpsimd) loads
                    # measured ~15% slower aggregate DMA on every core.
                    nc.sync.dma_start(out=a, in_=audio[r0 : r0 + P, c0 : c0 + D2])
                    nc.sync.dma_start(out=v, in_=visual[r0 : r0 + P, c0 : c0 + D2])
                    ah.append(a)
                    vh.append(v)
                    # s half on DVE; audio^2 rowsum half on ACT (full-size
                    # squares land in the dead visual half as scratch).
                    nc.vector.reduce_sum(
                        out=sh[:, h : h + 1], in_=v, axis=mybir.AxisListType.X
                    )
                    nc.scalar.activation(
                        out=v,
                        in_=a,
                        func=mybir.ActivationFunctionType.Square,
                        bias=zero,
                        accum_out=qh[:, h : h + 1],
                    )

                # sc = s / max(sqrt(s^2 * asq), eps)
                s = stats.tile([P, 1], FP32)
                nc.vector.tensor_add(out=s, in0=sh[:, 0:1], in1=sh[:, 1:2])
                nrm = stats.tile([P, 1], FP32)
                nc.vector.tensor_add(out=nrm, in0=qh[:, 0:1], in1=qh[:, 1:2])
                nc.vector.tensor_mul(out=nrm, in0=nrm, in1=s)
                nc.vector.tensor_mul(out=nrm, in0=nrm, in1=s)
                nc.scalar.activation(
                    out=nrm,
                    in_=nrm,
                    func=mybir.ActivationFunctionType.Sqrt,
                    bias=zero,
                )
                nc.vector.tensor_scalar_max(out=nrm, in0=nrm, scalar1=EPS)
                nc.vector.reciprocal(out=nrm, in_=nrm)
                sc = stats.tile([P, 1], FP32)
                nc.vector.tensor_mul(out=sc, in0=s, in1=nrm)

                # out halves = audio halves * sc, in place, on two engines
                # in parallel; store each half as soon as its mul is done,
                # alternating the two HWDGE rings.
                nc.scalar.mul(out=ah[0], in_=ah[0], mul=sc)
                nc.vector.tensor_scalar_mul(out=ah[1], in0=ah[1], scalar1=sc)
                nc.scalar.dma_start(out=out[r0 : r0 + P, 0:D2], in_=ah[0])
                nc.sync.dma_start(out=out[r0 : r0 + P, D2:D], in_=ah[1])

    # Runs Bacc.compile() (event-sem wait splitting, reg alloc, ISA
    # codegen) and freezes; run_bass_via_pjrt requires a finalized module.
    nc.finalize()
    return nc


_NC = None


def _get_nc():
    global _NC
    if _NC is None:
        _NC = build_bass()
    return _NC


def kernel(audio: np.ndarray, visual: np.ndarray) -> np.ndarray:
    audio = np.ascontiguousarray(audio, dtype=np.float32)
    visual = np.ascontiguousarray(visual, dtype=np.float32)
    nc = _get_nc()
    in_maps = [
        {
            "audio": audio[i * ROWS : (i + 1) * ROWS],
            "visual": visual[i * ROWS : (i + 1) * ROWS],
        }
        for i in range(N_CORES)
    ]
    res = run_bass_kernel_spmd(nc, in_maps, core_ids=list(range(N_CORES)))
    return np.concatenate([r["out"] for r in res.results], axis=0)
